# revision 6
# baseline (speedup 1.0000x reference)
"""Trainium2 Bass kernel for nn_Encoder_67138928771138 (CfC/LTC encoder).

Per time step: ncps mixed-memory LSTM cell (LATENT=512) followed by a
WiredCfCCell with 3 sequential sparse-masked CfC layers (inter/command/motor).
T=256 steps, B=128. Output = final (h, c), each (128, 512) f32.

Strategy (pure data parallel, 8 cores, B_local=16):
  - Fully transposed dataflow: features on SBUF partitions, batch (16) on the
    free dim.  All matmuls use weights as the stationary operand (lhsT) and
    activations [K<=128, 16] as the moving operand; PSUM accumulates fp32.
  - h is stored in a 6-block layout [128 part, 6*16 cols]: feature pieces
    (128, 88, 128, 15, 128, 25) at offsets (0,128,216,344,359,487) so every
    piece lives at partition base 0 of its own 16-col block.  inter=216,
    command=143, motor=153 boundaries all fall on piece edges, so the CfC
    layers slice h without any cross-partition copies.
  - LSTM bias (+1.0 on the forget gate) is folded into an extra "ones" row of
    the per-step input column, so z = [wi|bi]ᵀ-stationary @ [x;dt;1] + wr
    chunks @ h chunks accumulate entirely in PSUM.
  - CfC: ti = sigmoid(xc@(wb-wa).T + (bb-ba)) -> single fused weight wt.
    w1*mask / w2*mask premultiplied host-side (loop-invariant weight prep).
  - Pointwise runs on ACT (sigmoid/tanh, one shared table set) and DVE.

kernel(**inputs) takes FULL inputs, shards batch over 8 cores, runs via
run_bass_kernel_spmd, and reassembles full (h, c).
"""

import sys

sys.path.insert(0, "/opt/trn_rl_repo")

import numpy as np
import ml_dtypes
from contextlib import ExitStack

import concourse.bass as bass  # noqa: F401  (namespace import keeps reloads happy)
import concourse.bacc as bacc
import concourse.mybir as mybir
import concourse.tile as tile

# ---------------- problem constants (hardcoded per spec) ----------------
B, T, NV = 128, 256, 8
IN_DIM = NV + 1            # x ++ dt = 9
H = 512
G4 = 4 * H                 # 2048
MOTOR, COMMAND, INTER = 153, 143, 216
NCORES = 8
BL = B // NCORES           # 16

# 6-piece split of the 512 h-features: [inter c0, inter c1, cmd c0, cmd c1,
# motor c0, motor c1]
P6_OFF = [0, 128, 216, 344, 359, 487]
P6_SZ = [128, 88, 128, 15, 128, 25]
NJ = 6

OUT_L = [INTER, COMMAND, MOTOR]                     # 216 143 153
IN_L = [IN_DIM + INTER, INTER + COMMAND, COMMAND + MOTOR]  # 225 359 296
C1_L = [o - 128 for o in OUT_L]                     # 88 15 25
# K-chunk row splits of each CfC layer's input xc
KSPLIT = [
    [(0, 9), (9, 137), (137, 225)],
    [(0, 128), (128, 216), (216, 344), (344, 359)],
    [(0, 128), (128, 143), (143, 271), (271, 296)],
]

F32 = mybir.dt.float32
BF16 = mybir.dt.bfloat16
AF = mybir.ActivationFunctionType


def build_nc(dtype_mm=BF16, t_steps=T):
    """Build the per-core Bass/Tile program (identical on all cores)."""
    nc = bacc.Bacc("TRN2", target_bir_lowering=False, debug=False)

    np_mm = ml_dtypes.bfloat16 if dtype_mm == BF16 else np.float32

    xdt = nc.dram_tensor("xdt", [IN_DIM + 1, t_steps * BL], dtype_mm,
                         kind="ExternalInput")
    wit = nc.dram_tensor("wit", [IN_DIM + 1, G4], dtype_mm, kind="ExternalInput")
    wrt = nc.dram_tensor("wrt", [H, G4], dtype_mm, kind="ExternalInput")
    cfc_d = [
        nc.dram_tensor(f"cfc{l}", [IN_L[l], 3 * OUT_L[l]], dtype_mm,
                       kind="ExternalInput")
        for l in range(3)
    ]
    bt_d = [
        nc.dram_tensor(f"bt{l}", [128, 96], F32, kind="ExternalInput")
        for l in range(3)
    ]
    h_out = nc.dram_tensor("h_out", [128, 96], F32, kind="ExternalOutput")
    c_out = nc.dram_tensor("c_out", [128, 96], F32, kind="ExternalOutput")

    with ExitStack() as ctx:
        tc = ctx.enter_context(tile.TileContext(nc))
        const = ctx.enter_context(tc.tile_pool(name="const", bufs=1))
        state = ctx.enter_context(tc.tile_pool(name="state", bufs=2))
        work = ctx.enter_context(tc.tile_pool(name="work", bufs=3))
        psum = ctx.enter_context(tc.tile_pool(name="psum", bufs=2, space="PSUM"))

        # ---- load constants ----
        s_xdt = const.tile([IN_DIM + 1, t_steps * BL], dtype_mm, tag="xdt")
        nc.sync.dma_start(out=s_xdt, in_=xdt[:])
        s_wit = const.tile([IN_DIM + 1, G4], dtype_mm, tag="wit")
        nc.sync.dma_start(out=s_wit, in_=wit[:])
        s_wr = []
        for k in range(NJ):
            tl = const.tile([P6_SZ[k], G4], dtype_mm, tag=f"wr{k}")
            nc.sync.dma_start(out=tl, in_=wrt[P6_OFF[k]:P6_OFF[k] + P6_SZ[k], :])
            s_wr.append(tl)
        s_cfc = []
        for l in range(3):
            tiles = []
            for ki, (r0, r1) in enumerate(KSPLIT[l]):
                tl = const.tile([r1 - r0, 3 * OUT_L[l]], dtype_mm, tag=f"cfc{l}_{ki}")
                nc.sync.dma_start(out=tl, in_=cfc_d[l][r0:r1, :])
                tiles.append(tl)
            s_cfc.append(tiles)
        s_bt = []
        for l in range(3):
            tl = const.tile([128, 96], F32, tag=f"bt{l}")
            nc.sync.dma_start(out=tl, in_=bt_d[l][:])
            s_bt.append(tl)

        # ---- initial state ----
        h_prev = state.tile([128, 96], dtype_mm, tag="h")
        c_prev = state.tile([128, 96], F32, tag="c")
        nc.vector.memset(h_prev, 0.0)
        nc.vector.memset(c_prev, 0.0)

        h_fin = const.tile([128, 96], F32, tag="hfin")  # f32 copy of last h

        for t in range(t_steps):
            xcol = s_xdt[:, t * BL:(t + 1) * BL]          # [10, 16] incl ones row
            xcol9 = s_xdt[0:IN_DIM, t * BL:(t + 1) * BL]  # [9, 16] for CfC L0

            # ---------------- LSTM gates: zT in 24-piece layout ----------------
            zp = psum.tile([128, 384], F32, tag="zp")
            # junk rows of the piece layout are never matmul-written but are
            # read by the full-rect ACT ops; matmul start=True overwrites the
            # written region regardless
            nc.vector.memset(zp, 0.0)
            for g in range(4):
                for j in range(NJ):
                    w = P6_SZ[j]
                    col0 = H * g + P6_OFF[j]
                    o = zp[0:w, 16 * (6 * g + j):16 * (6 * g + j) + 16]
                    nc.tensor.matmul(o, s_wit[:, col0:col0 + w], xcol,
                                     start=True, stop=False)
                    for k in range(NJ):
                        nc.tensor.matmul(
                            o, s_wr[k][:, col0:col0 + w],
                            h_prev[0:P6_SZ[k], 16 * k:16 * k + 16],
                            start=False, stop=(k == NJ - 1))

            # ---------------- LSTM pointwise ----------------
            # gate blocks in zp: i=[0:96), ig=[96:192), fg=[192:288), og=[288:384)
            sig = work.tile([128, 288], F32, tag="sig")
            nc.scalar.activation(sig, zp[:, 96:384], AF.Sigmoid)
            tai = work.tile([128, 96], F32, tag="tai")
            nc.scalar.activation(tai, zp[:, 0:96], AF.Tanh)
            tmp = work.tile([128, 96], F32, tag="tmp")
            nc.vector.tensor_mul(tmp, tai, sig[:, 0:96])        # tanh(i)*sig(ig)
            c_new = state.tile([128, 96], F32, tag="c")
            nc.vector.tensor_mul(c_new, c_prev, sig[:, 96:192])  # c*sig(fg+1)
            nc.vector.tensor_add(c_new, c_new, tmp)
            tcc = work.tile([128, 96], F32, tag="tcc")
            nc.scalar.activation(tcc, c_new, AF.Tanh)
            hl = work.tile([128, 96], dtype_mm, tag="hl")        # h_lstm
            nc.vector.tensor_mul(hl, tcc, sig[:, 192:288])       # tanh(c)*sig(og)

            # ---------------- CfC layers ----------------
            last = t == t_steps - 1
            h_new = state.tile([128, 96], dtype_mm, tag="h")
            rhs_per_layer = [
                [xcol9, hl[0:128, 0:16], hl[0:88, 16:32]],
                [h_new[0:128, 0:16], h_new[0:88, 16:32],
                 hl[0:128, 32:48], hl[0:15, 48:64]],
                [h_new[0:128, 32:48], h_new[0:15, 48:64],
                 hl[0:128, 64:80], hl[0:25, 80:96]],
            ]
            for l in range(3):
                ol, c1 = OUT_L[l], C1_L[l]
                rhs_list = rhs_per_layer[l]
                cp = psum.tile([128, 96], F32, tag=f"cp{l}")
                nc.vector.memset(cp, 0.0)
                nk = len(rhs_list)
                for tau in range(3):
                    for cc in (0, 1):
                        w = 128 if cc == 0 else c1
                        o = cp[0:w, 16 * (2 * tau + cc):16 * (2 * tau + cc) + 16]
                        for ki, rhs in enumerate(rhs_list):
                            lhs = s_cfc[l][ki][:, tau * ol + 128 * cc:
                                               tau * ol + 128 * cc + w]
                            nc.tensor.matmul(o, lhs, rhs,
                                             start=ki == 0, stop=ki == nk - 1)
                # blocks in cp: ff1c0 ff1c1 ff2c0 ff2c1 tic0 tic1 (16 cols each)
                zc = work.tile([128, 96], F32, tag=f"zc{l}")
                nc.vector.tensor_add(zc, cp, s_bt[l])
                th = work.tile([128, 64], F32, tag=f"th{l}")
                nc.scalar.activation(th, zc[:, 0:64], AF.Tanh)
                sg = work.tile([128, 32], F32, tag=f"sg{l}")
                nc.scalar.activation(sg, zc[:, 64:96], AF.Sigmoid)
                d = work.tile([128, 32], F32, tag=f"d{l}")
                nc.vector.tensor_sub(d, th[:, 32:64], th[:, 0:32])  # ff2-ff1
                e = work.tile([128, 32], F32, tag=f"e{l}")
                nc.vector.tensor_mul(e, sg, d)                      # ti*(ff2-ff1)
                # out = ff1 + ti*(ff2-ff1) -> h_new blocks (2l, 2l+1)
                nc.vector.tensor_add(h_new[0:128, 32 * l:32 * l + 16],
                                     th[:, 0:16], e[:, 0:16])
                nc.vector.tensor_add(h_new[0:c1, 32 * l + 16:32 * l + 32],
                                     th[0:c1, 16:32], e[0:c1, 16:32])
                if last:
                    nc.vector.tensor_add(h_fin[0:128, 32 * l:32 * l + 16],
                                         th[:, 0:16], e[:, 0:16])
                    nc.vector.tensor_add(h_fin[0:c1, 32 * l + 16:32 * l + 32],
                                         th[0:c1, 16:32], e[0:c1, 16:32])

            h_prev, c_prev = h_new, c_new

        # ---- outputs: DMA only the valid rows of each block (outputs are
        # pre-zeroed, junk rows must not leak PSUM garbage) ----
        for j in range(NJ):
            sz = P6_SZ[j]
            nc.sync.dma_start(out=h_out[0:sz, 16 * j:16 * j + 16],
                              in_=h_fin[0:sz, 16 * j:16 * j + 16])
            nc.sync.dma_start(out=c_out[0:sz, 16 * j:16 * j + 16],
                              in_=c_prev[0:sz, 16 * j:16 * j + 16])

    nc.compile()
    return nc, np_mm


# ---------------- host-side input prep ----------------

def _prep_shared(inputs, np_mm):
    """Weight re-layout (pure per-parameter prep, no model compute)."""
    f = lambda a: np.asarray(a, np.float32)
    wi, wr, bi = f(inputs["lstm_wi"]), f(inputs["lstm_wr"]), f(inputs["lstm_bi"])
    bi_adj = bi.copy()
    bi_adj[2 * H:3 * H] += 1.0  # forget-gate +1
    wit = np.concatenate([wi, bi_adj[:, None]], 1).T.astype(np_mm)  # [10, 2048]
    wrt = wr.T.astype(np_mm)                                        # [512, 2048]

    masks = [f(inputs["m0"]), f(inputs["m1"]), f(inputs["m2"])]
    cfc, bt = [], []
    for l in range(3):
        w1 = f(inputs[f"w1_{l}"]) * masks[l]
        w2 = f(inputs[f"w2_{l}"]) * masks[l]
        wt = f(inputs[f"wb_{l}"]) - f(inputs[f"wa_{l}"])
        cfc.append(np.concatenate([w1.T, w2.T, wt.T], 1).astype(np_mm))
        biases = [f(inputs[f"b1_{l}"]), f(inputs[f"b2_{l}"]),
                  f(inputs[f"bb_{l}"]) - f(inputs[f"ba_{l}"])]
        tile_b = np.zeros((128, 96), np.float32)
        ol = OUT_L[l]
        for tau in range(3):
            for cc in (0, 1):
                w = 128 if cc == 0 else ol - 128
                col = 16 * (2 * tau + cc)
                tile_b[0:w, col:col + 16] = biases[tau][128 * cc:128 * cc + w][:, None]
        bt.append(tile_b)
    return wit, wrt, cfc, bt


def _prep_xdt(inputs, core, np_mm, t_steps=T):
    x = np.asarray(inputs["x"], np.float32)[:, :t_steps]
    dt = np.asarray(inputs["dt"], np.float32)[:, :t_steps]
    b0 = core * BL
    xc = np.concatenate([x, dt], -1)[b0:b0 + BL]          # [16, T, 9]
    xc = xc.transpose(1, 2, 0)                            # [T, 9, 16]
    ones = np.ones((t_steps, 1, BL), np.float32)
    arr = np.concatenate([xc, ones], 1)                   # [T, 10, 16]
    return arr.transpose(1, 0, 2).reshape(IN_DIM + 1, t_steps * BL).astype(np_mm)


def _unpack_state(out_tile):
    """[128, 96] block layout -> [BL, 512]."""
    res = np.zeros((BL, H), np.float32)
    for j in range(NJ):
        sz = P6_SZ[j]
        res[:, P6_OFF[j]:P6_OFF[j] + sz] = out_tile[0:sz, 16 * j:16 * j + 16].T
    return res


_CACHE = {}


def _get_nc(dtype_mm=BF16, t_steps=T):
    key = (dtype_mm, t_steps)
    if key not in _CACHE:
        _CACHE[key] = build_nc(dtype_mm, t_steps)
    return _CACHE[key]


def kernel(**inputs):
    from concourse.bass_utils import run_bass_kernel_spmd

    nc, np_mm = _get_nc()
    wit, wrt, cfc, bt = _prep_shared(inputs, np_mm)
    shared = {"wit": wit, "wrt": wrt,
              "cfc0": cfc[0], "cfc1": cfc[1], "cfc2": cfc[2],
              "bt0": bt[0], "bt1": bt[1], "bt2": bt[2]}
    in_maps = [dict(shared, xdt=_prep_xdt(inputs, c, np_mm)) for c in range(NCORES)]
    res = run_bass_kernel_spmd(nc, in_maps, core_ids=list(range(NCORES))).results
    h = np.concatenate([_unpack_state(res[c]["h_out"]) for c in range(NCORES)], 0)
    c = np.concatenate([_unpack_state(res[c]["c_out"]) for c in range(NCORES)], 0)
    return h, c
